# revision 1
# baseline (speedup 1.0000x reference)
"""ASPPModulatedDeformableC3D on 8 Trainium2 NeuronCores.

Strategy: every heavy stage of the network is a GEMM over the 18432
spatial positions (B=1, D=8, H=48, W=48). One generic Bass GEMM kernel
(K<=6912, M<=1280, N=2304 per core, bf16 inputs, fp32 accumulate) is
compiled once and invoked four times, with positions split 8-way across
the cores (2304 each); weights are replicated. Host numpy does the
im2col packing, bias/relu/sigmoid, and the trilinear sampling packing
between stages.

Set KERNEL_FAKE_GEMM=1 to replace the device GEMM with numpy (host-only
validation of the surrounding math).
"""
import os

import numpy as np

N_CORES = 8
B, CI, D, H, W = 1, 16, 8, 48, 48
NPOS = D * H * W            # 18432
NPC = NPOS // N_CORES       # 2304 positions per core
MMAX = 1280
N_HALF = NPC // 2           # 1152
M_HALF = MMAX // 2          # 640

_FAKE = bool(int(os.environ.get("KERNEL_FAKE_GEMM", "0")))
_NC = {}                    # kch -> compiled Bass


def _build_gemm(KCH):
    """C[M,N] = A[K,M]^T @ B[K,N]; A,B bf16 in DRAM, C fp32."""
    from contextlib import ExitStack
    import concourse.tile as tile
    from concourse import bacc, mybir

    nc = bacc.Bacc("TRN2", target_bir_lowering=False, debug=False,
                   enable_asserts=False, num_devices=N_CORES)
    a = nc.dram_tensor("a", [KCH, 128, MMAX], mybir.dt.bfloat16,
                       kind="ExternalInput").ap()
    b = nc.dram_tensor("b", [KCH, 128, NPC], mybir.dt.bfloat16,
                       kind="ExternalInput").ap()
    c = nc.dram_tensor("c", [MMAX, NPC], mybir.dt.float32,
                       kind="ExternalOutput").ap()

    with tile.TileContext(nc) as tc:
        with ExitStack() as ctx:
            bpool = ctx.enter_context(tc.tile_pool(name="bp", bufs=1))
            apool = ctx.enter_context(tc.tile_pool(name="ap", bufs=1))
            opool = ctx.enter_context(tc.tile_pool(name="op", bufs=3))
            pspool = ctx.enter_context(
                tc.tile_pool(name="ps", bufs=4, space="PSUM"))
            for nh in range(2):
                tb = bpool.tile([128, KCH, N_HALF], mybir.dt.bfloat16)
                nc.sync.dma_start(
                    tb[:], b[:, :, nh * N_HALF:(nh + 1) * N_HALF]
                    .rearrange("k p n -> p k n"))
                for mh in range(2):
                    ta = apool.tile([128, KCH, M_HALF], mybir.dt.bfloat16)
                    nc.sync.dma_start(
                        ta[:], a[:, :, mh * M_HALF:(mh + 1) * M_HALF]
                        .rearrange("k p m -> p k m"))
                    for m in range(5):          # 5 x 128 = 640 rows of C
                        to = opool.tile([128, N_HALF], mybir.dt.float32)
                        for n in range(3):      # 3 x 384 = 1152 cols
                            ps = pspool.tile([128, 384], mybir.dt.float32)
                            for k in range(KCH):
                                nc.tensor.matmul(
                                    ps[:],
                                    ta[:, k, m * 128:(m + 1) * 128],
                                    tb[:, k, n * 384:(n + 1) * 384],
                                    start=(k == 0), stop=(k == KCH - 1))
                            nc.vector.tensor_copy(
                                to[:, n * 384:(n + 1) * 384], ps[:])
                        nc.sync.dma_start(
                            c[mh * M_HALF + m * 128:
                              mh * M_HALF + (m + 1) * 128,
                              nh * N_HALF:(nh + 1) * N_HALF], to[:])
    nc.compile()
    return nc


def _gemm(A, Bm):
    """A [K, M] f32, Bm [K, 18432] f32 -> [M, 18432] f32 via 8-core SPMD."""
    if _FAKE:
        return A.T.astype(np.float32) @ Bm.astype(np.float32)
    from concourse.bass_utils import run_bass_kernel_spmd
    import ml_dtypes
    K, M = A.shape
    KCH = 54 if K > 2048 else 16    # two compiled variants
    if KCH not in _NC:
        _NC[KCH] = _build_gemm(KCH)
    Ap = np.zeros((KCH * 128, MMAX), np.float32)
    Ap[:K, :M] = A
    Ap = Ap.reshape(KCH, 128, MMAX).astype(ml_dtypes.bfloat16)
    Bp = np.zeros((KCH * 128, NPOS), np.float32)
    Bp[:K] = Bm
    Bp = Bp.reshape(KCH, 128, NPOS).astype(ml_dtypes.bfloat16)
    ins = [{"a": Ap, "b": np.ascontiguousarray(Bp[:, :, i * NPC:(i + 1) * NPC])}
           for i in range(N_CORES)]
    res = run_bass_kernel_spmd(_NC[KCH], ins, core_ids=list(range(N_CORES)))
    out = np.concatenate([res.results[i]["c"] for i in range(N_CORES)], axis=1)
    return out[:M]


def _im2col(v, dil):
    """v [C, D, H, W] -> [27*C, NPOS], tap-major, zero padded, dilation dil."""
    C = v.shape[0]
    p = dil
    vp = np.pad(v, ((0, 0), (p, p), (p, p), (p, p)))
    rows = []
    for kz in (-1, 0, 1):
        for ky in (-1, 0, 1):
            for kx in (-1, 0, 1):
                rows.append(vp[:, p + kz * dil:p + kz * dil + D,
                               p + ky * dil:p + ky * dil + H,
                               p + kx * dil:p + kx * dil + W]
                            .reshape(C, NPOS))
    return np.concatenate(rows, axis=0)


def _wflat(w):
    """w [O, C, 3,3,3] -> [27*C, O] matching _im2col row order."""
    O, C = w.shape[:2]
    return w.reshape(O, C, 27).transpose(2, 1, 0).reshape(27 * C, O)


def _trilinear_modulated(x, offsets, alpha):
    """Exact numpy port of reference trilinear sampling; returns
    col [27*16, NPOS] with col[(k,c)] = alpha_k * sample_k(x)_c."""
    xc = x[0].transpose(1, 2, 3, 0)                      # [D,H,W,C]
    off = offsets[0].reshape(27, 3, D, H, W)
    alpha = alpha[0]                                     # [27, D, H, W]
    zz, yy, xx = np.meshgrid(np.arange(D), np.arange(H), np.arange(W),
                             indexing="ij")
    base = np.stack([zz, yy, xx]).astype(np.float32)     # [3, D, H, W]
    cols = np.empty((27, CI, NPOS), np.float32)
    k = 0
    for kz in (-1, 0, 1):
        for ky in (-1, 0, 1):
            for kx in (-1, 0, 1):
                koff = np.array([kz, ky, kx], np.float32)
                p = base + koff[:, None, None, None] + off[k]
                pz, py, px = p[0], p[1], p[2]
                z0 = np.floor(pz); y0 = np.floor(py); x0 = np.floor(px)
                fz = pz - z0; fy = py - y0; fx = px - x0
                z0 = z0.astype(np.int64); y0 = y0.astype(np.int64)
                x0 = x0.astype(np.int64)
                acc = np.zeros((D, H, W, CI), np.float32)
                for dz in (0, 1):
                    for dy in (0, 1):
                        for dx in (0, 1):
                            zi = z0 + dz; yi = y0 + dy; xi = x0 + dx
                            valid = ((zi >= 0) & (zi < D) & (yi >= 0)
                                     & (yi < H) & (xi >= 0) & (xi < W))
                            wz = fz if dz else (1.0 - fz)
                            wy = fy if dy else (1.0 - fy)
                            wx = fx if dx else (1.0 - fx)
                            wgt = wz * wy * wx * valid.astype(np.float32)
                            val = xc[np.clip(zi, 0, D - 1),
                                     np.clip(yi, 0, H - 1),
                                     np.clip(xi, 0, W - 1)]
                            acc += val * wgt[..., None]
                cols[k] = (acc * alpha[k][..., None]).transpose(3, 0, 1, 2) \
                    .reshape(CI, NPOS)
                k += 1
    return cols.reshape(27 * CI, NPOS)


def kernel(x, w1, b1, w2, b2, w3, b3, w4, b4, wg, bg, wp, bp,
           wdef, bdef, wdc, bdc):
    x = np.asarray(x, np.float32)
    xv = x[0]                                            # [16, D, H, W]
    xf = xv.reshape(CI, NPOS)

    # ---- stage 1: all ASPP branches in one GEMM (K = 16+432*3+16 = 1328)
    g = xv.mean(axis=(1, 2, 3))                          # [16]
    B1 = np.concatenate([
        xf,                                              # 1x1 branch
        _im2col(xv, 6), _im2col(xv, 12), _im2col(xv, 18),
        np.broadcast_to(g[:, None], (CI, NPOS)),         # global branch
    ], axis=0)                                           # [1328, NPOS]
    A1 = np.zeros((1328, 1280), np.float32)
    A1[0:16, 0:256] = w1.reshape(256, 16).T
    A1[16:448, 256:512] = _wflat(w2)
    A1[448:880, 512:768] = _wflat(w3)
    A1[880:1312, 768:1024] = _wflat(w4)
    A1[1312:1328, 1024:1280] = wg.reshape(256, 16).T
    cat = _gemm(A1, B1)                                  # [1280, NPOS]
    bias1 = np.concatenate([b1, b2, b3, b4, bg])
    cat = np.maximum(cat + bias1[:, None], 0.0)

    # ---- stage 2: projection 1280 -> 256
    pyr = _gemm(wp.reshape(256, 1280).T, cat)
    pyr = np.maximum(pyr + np.asarray(bp)[:, None], 0.0) # [256, NPOS]

    # ---- stage 3: offset/alpha conv (3x3x3 pad 1 on pyramid)
    B3 = _im2col(pyr.reshape(256, D, H, W), 1)           # [6912, NPOS]
    defo = _gemm(_wflat(wdef), B3) + np.asarray(bdef)[:, None]
    offsets = defo[:81].reshape(1, 81, D, H, W)
    alpha = 1.0 / (1.0 + np.exp(-defo[81:108]))
    alpha = alpha.reshape(1, 27, D, H, W)

    # ---- stage 4: modulated deformable conv
    col = _trilinear_modulated(x, offsets, alpha)        # [432, NPOS]
    out = _gemm(_wflat(wdc), col) + np.asarray(bdc)[:, None]
    return out.reshape(1, 32, D, H, W).astype(np.float32)



# revision 3
# speedup vs baseline: 17.7055x; 17.7055x over previous
"""ASPPModulatedDeformableC3D on 8 Trainium2 NeuronCores.

Single fused device dispatch computes ASPP (all dilated branches packed
into one K=736 GEMM; global-pool branch folded into the stage-2 bias),
the 1280->256 projection, and the 3x3x3 offset conv. Every core
redundantly computes the full pyramid (compute is ~2ms, wire is the
bottleneck), writes it to device DRAM, then indirect-DMA-gathers its own
z-slice +-1 slab (per-core index input; OOB rows stay zero, giving exact
conv z-padding) and produces defo[108, 2304] for its slice.

Weights are embedded in the NEFF via inline_tensor, so the warm-call
wire traffic is only x (bf16) + a tiny x-dependent bias in, defo out.
The data-dependent trilinear sampling and the final 432x32 GEMM run on
host (cheap; no efficient device gather at this granularity).

KERNEL_FAKE_GEMM=1 emulates the device program in numpy (host-only
validation of layouts/math).
"""
import os

import numpy as np

N_CORES = 8
CI, D, H, W = 16, 8, 48, 48
NPC = H * W                  # 2304 positions per z-slice (one core each)
NPOS = D * NPC
MID = 256
M1 = 1024                    # cat rows (4 branches; global folded into bias2)
K1T = 6                      # stage-1 K tiles (736 rows used, 768 padded)
K2T = 8                      # stage-2 K tiles (1024)
K3T = 54                     # stage-3 K tiles (6912 = 27 taps * 256)
NCH = [(0, 512), (512, 512), (1024, 512), (1536, 512), (2048, 256)]

_FAKE = bool(int(os.environ.get("KERNEL_FAKE_GEMM", "0")))
_STATE = {}


def _slots():
    """B1/A1 row layout: list of (row0, dil, kz, ky, kx). Slot 0 is the
    1x1 branch; d12/d18 kz=+-1 taps are always out of z-bounds (D=8) and
    are omitted entirely."""
    out = [(0, 0, 0, 0, 0)]
    r = 16
    for d, kz in [(6, 0), (12, 0), (18, 0), (6, -1), (6, 1)]:
        for ky in (-1, 0, 1):
            for kx in (-1, 0, 1):
                out.append((r, d, kz, ky, kx))
                r += 16
    assert r == 736
    return out


_SLOTS = _slots()
_BRANCH = {0: 0, 6: 1, 12: 2, 18: 3}


def _pack_weights(w1, w2, w3, w4, wp, wdef, b1, b2, b3, b4, bdef):
    wb = {6: np.asarray(w2, np.float32), 12: np.asarray(w3, np.float32),
          18: np.asarray(w4, np.float32)}
    A1 = np.zeros((768, M1), np.float32)
    A1[0:16, 0:256] = np.asarray(w1, np.float32).reshape(256, 16).T
    for (r0, d, kz, ky, kx) in _SLOTS[1:]:
        A1[r0:r0 + 16, 256 * _BRANCH[d]:256 * (_BRANCH[d] + 1)] = \
            wb[d][:, :, kz + 1, ky + 1, kx + 1].T
    a1 = A1.reshape(6, 128, M1).transpose(1, 0, 2).reshape(128, 6 * M1)

    WpT = np.asarray(wp, np.float32).reshape(256, 1280)[:, :1024].T
    a2 = WpT.reshape(8, 128, 256).transpose(1, 0, 2).reshape(128, 8 * 256)

    A3 = np.asarray(wdef, np.float32).reshape(108, 256, 27) \
        .transpose(2, 1, 0).reshape(6912, 108)
    a3 = A3.reshape(54, 128, 108).transpose(1, 0, 2).reshape(128, 54 * 108)

    bias1 = np.concatenate([np.asarray(b, np.float32) for b in (b1, b2, b3, b4)])
    b1i = bias1.reshape(8, 128).T.copy()
    bdefi = np.zeros((128, 1), np.float32)
    bdefi[:108, 0] = np.asarray(bdef, np.float32)
    import ml_dtypes
    bf = ml_dtypes.bfloat16
    return {"a1": a1.astype(bf), "a2": a2.astype(bf), "a3": a3.astype(bf),
            "b1": b1i, "bdef": bdefi}


def _build_nc(pk):
    from contextlib import ExitStack
    import concourse.tile as tile
    from concourse import bacc, bass, mybir

    nc = bacc.Bacc("TRN2", target_bir_lowering=False, debug=False,
                   enable_asserts=False, num_devices=N_CORES)
    bf16 = mybir.dt.bfloat16
    f32 = mybir.dt.float32
    xin = nc.dram_tensor("xin", [CI, D, H, W], bf16, kind="ExternalInput").ap()
    b2in = nc.dram_tensor("b2in", [128, 2], f32, kind="ExternalInput").ap()
    gidx = nc.dram_tensor("gidx", [128, 6], mybir.dt.int32,
                          kind="ExternalInput").ap()
    defo = nc.dram_tensor("defo", [108, NPC], f32, kind="ExternalOutput").ap()
    pyrd_h = nc.dram_tensor("pyrd", [2048, NPC], bf16, kind="Internal")
    pyrd = pyrd_h.ap()

    a1d = nc.inline_tensor(pk["a1"], "a1w").ap()
    a2d = nc.inline_tensor(pk["a2"], "a2w").ap()
    a3d = nc.inline_tensor(pk["a3"], "a3w").ap()
    b1d = nc.inline_tensor(pk["b1"], "b1w").ap()
    bdd = nc.inline_tensor(pk["bdef"], "bdw").ap()

    with tile.TileContext(nc) as tc:
        with ExitStack() as ctx:
            wpool = ctx.enter_context(tc.tile_pool(name="w", bufs=1))
            b1pool = ctx.enter_context(tc.tile_pool(name="b1", bufs=7))
            catpool = ctx.enter_context(tc.tile_pool(name="cat", bufs=9))
            pyrpool = ctx.enter_context(tc.tile_pool(name="pyr", bufs=4))
            slabpool = ctx.enter_context(tc.tile_pool(name="slab", bufs=6))
            b3pool = ctx.enter_context(tc.tile_pool(name="b3", bufs=3))
            opool = ctx.enter_context(tc.tile_pool(name="o", bufs=1))
            ps12 = ctx.enter_context(tc.tile_pool(name="ps12", bufs=3,
                                                  space="PSUM"))
            ps3 = ctx.enter_context(tc.tile_pool(name="ps3", bufs=5,
                                                 space="PSUM"))

            a1s = wpool.tile([128, 6 * M1], bf16, tag="a1s")
            a2s = wpool.tile([128, 8 * 256], bf16, tag="a2s")
            a3s = wpool.tile([128, 54 * 108], bf16, tag="a3s")
            b1s = wpool.tile([128, 8], f32, tag="b1s")
            b2s = wpool.tile([128, 2], f32, tag="b2s")
            bds = wpool.tile([128, 1], f32, tag="bds")
            gis = wpool.tile([128, 6], mybir.dt.int32, tag="gis")
            nc.sync.dma_start(a1s[:], a1d)
            nc.sync.dma_start(a2s[:], a2d)
            nc.sync.dma_start(a3s[:], a3d)
            nc.sync.dma_start(b1s[:], b1d)
            nc.sync.dma_start(b2s[:], b2in)
            nc.sync.dma_start(bds[:], bdd)
            nc.sync.dma_start(gis[:], gidx)
            a1v = a1s[:].rearrange("p (k m) -> p k m", k=6)
            a2v = a2s[:].rearrange("p (k m) -> p k m", k=8)
            a3v = a3s[:].rearrange("p (k m) -> p k m", k=54)

            for z in range(D):
                b1t = [b1pool.tile([128, NPC], bf16, tag="b1t", name="b1t")
                       for _ in range(K1T)]
                for t in b1t:
                    nc.vector.memset(t[:], 0)
                for (r0, d, kz, ky, kx) in _SLOTS:
                    zin = z + kz * d
                    if not (0 <= zin < D):
                        continue
                    ys, ye = max(0, -ky * d), H - max(0, ky * d)
                    xs, xe = max(0, -kx * d), W - max(0, kx * d)
                    if ys >= ye or xs >= xe:
                        continue
                    kt, po = divmod(r0, 128)
                    dst = b1t[kt][po:po + 16, :] \
                        .rearrange("p (y x) -> p y x", y=H)[:, ys:ye, xs:xe]
                    src = xin[:, zin, ys + ky * d:ye + ky * d,
                              xs + kx * d:xe + kx * d]
                    nc.sync.dma_start(dst, src)

                catt = [catpool.tile([128, NPC], bf16, tag="catt", name="catt")
                        for _ in range(K2T)]
                for mt in range(8):
                    for (n0, nw) in NCH:
                        ps = ps12.tile([128, 512], f32, tag="ps")
                        for kt in range(K1T):
                            nc.tensor.matmul(
                                ps[:, :nw],
                                a1v[:, kt, mt * 128:(mt + 1) * 128],
                                b1t[kt][:, n0:n0 + nw],
                                start=(kt == 0), stop=(kt == K1T - 1))
                        nc.scalar.activation(
                            catt[mt][:, n0:n0 + nw], ps[:, :nw],
                            mybir.ActivationFunctionType.Relu,
                            bias=b1s[:, mt:mt + 1], scale=1.0)

                for mt2 in range(2):
                    pyrt = pyrpool.tile([128, NPC], bf16, tag="pyrt")
                    for (n0, nw) in NCH:
                        ps = ps12.tile([128, 512], f32, tag="ps")
                        for kt in range(K2T):
                            nc.tensor.matmul(
                                ps[:, :nw],
                                a2v[:, kt, mt2 * 128:(mt2 + 1) * 128],
                                catt[kt][:, n0:n0 + nw],
                                start=(kt == 0), stop=(kt == K2T - 1))
                        nc.scalar.activation(
                            pyrt[:, n0:n0 + nw], ps[:, :nw],
                            mybir.ActivationFunctionType.Relu,
                            bias=b2s[:, mt2:mt2 + 1], scale=1.0)
                    nc.sync.dma_start(
                        pyrd[z * 256 + mt2 * 128:z * 256 + (mt2 + 1) * 128, :],
                        pyrt[:])

            # gather own z-1..z+1 pyramid slab (OOB rows remain zero)
            st = [slabpool.tile([128, NPC], bf16, tag="st", name="st") for _ in range(6)]
            for s in range(6):
                nc.vector.memset(st[s][:], 0)
                nc.gpsimd.indirect_dma_start(
                    out=st[s][:], out_offset=None, in_=pyrd,
                    in_offset=bass.IndirectOffsetOnAxis(
                        ap=gis[:, s:s + 1], axis=0),
                    bounds_check=2047, oob_is_err=False)

            pst = [ps3.tile([128, 512], f32, tag="pst", name="pst") for _ in range(5)]
            for t in range(27):
                kz, r = divmod(t, 9)
                ky, kx = divmod(r, 3)
                kz, ky, kx = kz - 1, ky - 1, kx - 1
                ys, ye = max(0, -ky), H - max(0, ky)
                xs, xe = max(0, -kx), W - max(0, kx)
                for ct in range(2):
                    ktg = 2 * t + ct
                    b3 = b3pool.tile([128, NPC], bf16, tag="b3")
                    if ky or kx:
                        nc.vector.memset(b3[:], 0)
                    dst = b3[:].rearrange("p (y x) -> p y x", y=H)[:, ys:ye, xs:xe]
                    src = st[(kz + 1) * 2 + ct][:] \
                        .rearrange("p (y x) -> p y x", y=H)[:, ys + ky:ye + ky,
                                                            xs + kx:xe + kx]
                    nc.vector.tensor_copy(dst, src)
                    for ci, (n0, nw) in enumerate(NCH):
                        nc.tensor.matmul(
                            pst[ci][:108, :nw], a3v[:, ktg, :108],
                            b3[:, n0:n0 + nw],
                            start=(ktg == 0), stop=(ktg == K3T - 1))
            dfs = opool.tile([128, NPC], f32, tag="dfs")
            for ci, (n0, nw) in enumerate(NCH):
                nc.scalar.activation(
                    dfs[:108, n0:n0 + nw], pst[ci][:108, :nw],
                    mybir.ActivationFunctionType.Identity,
                    bias=bds[:108, 0:1], scale=1.0)
            nc.sync.dma_start(defo, dfs[:108, :])
    nc.compile()
    return nc


def _fake_device(x_bf, b2c, pk):
    """Numpy emulation of the device program (per-core loop), for layout
    validation."""
    A1 = pk["a1"].astype(np.float32).reshape(128, 6, M1) \
        .transpose(1, 0, 2).reshape(768, M1)
    A2 = pk["a2"].astype(np.float32).reshape(128, 8, 256) \
        .transpose(1, 0, 2).reshape(1024, 256)
    A3 = pk["a3"].astype(np.float32).reshape(128, 54, 108) \
        .transpose(1, 0, 2).reshape(6912, 108)
    bias1 = pk["b1"].T.reshape(1024)
    bdef = pk["bdef"][:108, 0]
    bp = b2c.T.reshape(256)
    x = x_bf.astype(np.float32)
    pyr = np.zeros((D, 256, NPC), np.float32)
    for z in range(D):
        B1 = np.zeros((768, NPC), np.float32)
        for (r0, d, kz, ky, kx) in _SLOTS:
            zin = z + kz * d
            if not (0 <= zin < D):
                continue
            ys, ye = max(0, -ky * d), H - max(0, ky * d)
            xs, xe = max(0, -kx * d), W - max(0, kx * d)
            blk = np.zeros((16, H, W), np.float32)
            blk[:, ys:ye, xs:xe] = x[:, zin, ys + ky * d:ye + ky * d,
                                     xs + kx * d:xe + kx * d]
            B1[r0:r0 + 16] = blk.reshape(16, NPC)
        cat = np.maximum(A1.T @ B1 + bias1[:, None], 0.0)
        pyr[z] = np.maximum(A2.T @ cat + bp[:, None], 0.0)
    defs = []
    for i in range(N_CORES):
        B3 = np.zeros((6912, NPC), np.float32)
        for t in range(27):
            kz, r = divmod(t, 9)
            ky, kx = divmod(r, 3)
            kz, ky, kx = kz - 1, ky - 1, kx - 1
            gz = i + kz
            if not (0 <= gz < D):
                continue
            ys, ye = max(0, -ky), H - max(0, ky)
            xs, xe = max(0, -kx), W - max(0, kx)
            blk = np.zeros((256, H, W), np.float32)
            blk[:, ys:ye, xs:xe] = pyr[gz].reshape(256, H, W)[
                :, ys + ky:ye + ky, xs + kx:xe + kx]
            B3[t * 256:(t + 1) * 256] = blk.reshape(256, NPC)
        defs.append(A3.T @ B3 + bdef[:, None])
    return defs


def _run_device(x_bf, b2c, pk):
    if _FAKE:
        return _fake_device(x_bf, b2c, pk)
    from concourse.bass_utils import run_bass_kernel_spmd
    if "nc" not in _STATE:
        _STATE["nc"] = _build_nc(pk)
    ins = []
    for i in range(N_CORES):
        gi = np.full((128, 6), 1 << 20, np.int32)
        for s in range(6):
            gz = i - 1 + s // 2
            if 0 <= gz < D:
                gi[:, s] = gz * 256 + (s % 2) * 128 + np.arange(128)
        ins.append({"xin": x_bf, "b2in": b2c, "gidx": gi})
    res = run_bass_kernel_spmd(_STATE["nc"], ins, core_ids=list(range(N_CORES)))
    return [res.results[i]["defo"] for i in range(N_CORES)]


def _trilinear_modulated(x, offsets, alpha):
    """col[(k,c), pos] = alpha_k * trilinear_sample_k(x)_c; exact port of
    the reference DCN sampling, vectorized over taps."""
    xc = x[0].transpose(1, 2, 3, 0)                      # [D,H,W,C]
    off = offsets[0].reshape(27, 3, D, H, W)
    alpha = alpha[0]
    zz, yy, xx = np.meshgrid(np.arange(D), np.arange(H), np.arange(W),
                             indexing="ij")
    base = np.stack([zz, yy, xx]).astype(np.float32)[None]   # [1,3,D,H,W]
    kg = np.stack(np.meshgrid(*([np.arange(-1, 2)] * 3), indexing="ij"), -1)
    kgrid = kg.reshape(27, 3).astype(np.float32)
    p = base + kgrid[:, :, None, None, None] + off           # [27,3,D,H,W]
    pz, py, px = p[:, 0], p[:, 1], p[:, 2]
    z0 = np.floor(pz); y0 = np.floor(py); x0 = np.floor(px)
    fz = pz - z0; fy = py - y0; fx = px - x0
    z0 = z0.astype(np.int64); y0 = y0.astype(np.int64); x0 = x0.astype(np.int64)
    acc = np.zeros((27, D, H, W, CI), np.float32)
    xf = xc.reshape(NPOS, CI)
    for dz in (0, 1):
        for dy in (0, 1):
            for dx in (0, 1):
                zi = z0 + dz; yi = y0 + dy; xi = x0 + dx
                valid = ((zi >= 0) & (zi < D) & (yi >= 0) & (yi < H)
                         & (xi >= 0) & (xi < W))
                wz = fz if dz else (1.0 - fz)
                wy = fy if dy else (1.0 - fy)
                wx = fx if dx else (1.0 - fx)
                wgt = (wz * wy * wx * valid).astype(np.float32)
                lin = (np.clip(zi, 0, D - 1) * H + np.clip(yi, 0, H - 1)) * W \
                    + np.clip(xi, 0, W - 1)
                acc += xf[lin] * wgt[..., None]
    acc *= alpha[..., None]
    return acc.transpose(0, 4, 1, 2, 3).reshape(27 * CI, NPOS)


def kernel(x, w1, b1, w2, b2, w3, b3, w4, b4, wg, bg, wp, bp,
           wdef, bdef, wdc, bdc):
    import ml_dtypes
    x = np.asarray(x, np.float32)
    fp = sum(float(np.sum(np.asarray(a))) for a in
             (w1, w2, w3, w4, wp, wdef, b1, b2, b3, b4, bdef))
    if _STATE.get("fp") != fp:
        _STATE.clear()
        _STATE["fp"] = fp
        _STATE["pk"] = _pack_weights(w1, w2, w3, w4, wp, wdef,
                                     b1, b2, b3, b4, bdef)
    pk = _STATE["pk"]

    g = x[0].mean(axis=(1, 2, 3))
    brg = np.maximum(np.asarray(bg, np.float32)
                     + np.asarray(wg, np.float32).reshape(256, 16) @ g, 0.0)
    bp2 = np.asarray(bp, np.float32) \
        + np.asarray(wp, np.float32).reshape(256, 1280)[:, 1024:] @ brg
    b2c = np.ascontiguousarray(bp2.reshape(2, 128).T)

    x_bf = x[0].astype(ml_dtypes.bfloat16)
    defs = _run_device(x_bf, b2c, pk)
    defo = np.stack([np.asarray(d, np.float32) for d in defs], axis=1) \
        .reshape(108, NPOS)                               # [108, D*H*W]

    offsets = defo[:81].reshape(1, 81, D, H, W)
    alpha = 1.0 / (1.0 + np.exp(-defo[81:108])).reshape(1, 27, D, H, W)
    col = _trilinear_modulated(x, offsets, alpha)          # [432, NPOS]
    wdcf = np.asarray(wdc, np.float32).reshape(32, 16, 27) \
        .transpose(2, 1, 0).reshape(432, 32)
    out = wdcf.T @ col + np.asarray(bdc, np.float32)[:, None]
    return out.reshape(1, 32, D, H, W).astype(np.float32)


# revision 4
# speedup vs baseline: 24.8133x; 1.4014x over previous
"""ASPPModulatedDeformableC3D on 8 Trainium2 NeuronCores.

Single fused device dispatch computes ASPP (all dilated branches packed
into one K=736 GEMM; global-pool branch folded into the stage-2 bias),
the 1280->256 projection, and the 3x3x3 offset conv. Every core
redundantly computes the full pyramid (compute is ~2ms, wire is the
bottleneck), writes it to device DRAM, then indirect-DMA-gathers its own
z-slice +-1 slab (per-core index input; OOB rows stay zero, giving exact
conv z-padding) and produces defo[108, 2304] bf16 for its slice.

Weights are embedded in the NEFF via inline_tensor, so the warm-call
wire traffic is only x (bf16) + a tiny x-dependent bias in, defo out.
The dispatch goes through a cached jit(shard_map(bass_exec)) built once
(run_bass_kernel_spmd re-traces every call; that alone costs ~1s).
The data-dependent trilinear sampling and the final 432x32 GEMM run on
host under a jax-CPU jit (cheap; no efficient device gather at this
granularity).

KERNEL_FAKE_GEMM=1 emulates the device program in numpy.
KERNEL_V1=1 forces the run_bass_kernel_spmd dispatch path.
"""
import os

import numpy as np

N_CORES = 8
CI, D, H, W = 16, 8, 48, 48
NPC = H * W                  # 2304 positions per z-slice (one core each)
NPOS = D * NPC
MID = 256
M1 = 1024                    # cat rows (4 branches; global folded into bias2)
K1T = 6                      # stage-1 K tiles (736 rows used, 768 padded)
K2T = 8                      # stage-2 K tiles (1024)
K3T = 54                     # stage-3 K tiles (6912 = 27 taps * 256)
NCH = [(0, 512), (512, 512), (1024, 512), (1536, 512), (2048, 256)]

_FAKE = bool(int(os.environ.get("KERNEL_FAKE_GEMM", "0")))
_V1 = bool(int(os.environ.get("KERNEL_V1", "0")))
_STATE = {}


def _slots():
    """B1/A1 row layout: list of (row0, dil, kz, ky, kx). Slot 0 is the
    1x1 branch; d12/d18 kz=+-1 taps are always out of z-bounds (D=8) and
    are omitted entirely."""
    out = [(0, 0, 0, 0, 0)]
    r = 16
    for d, kz in [(6, 0), (12, 0), (18, 0), (6, -1), (6, 1)]:
        for ky in (-1, 0, 1):
            for kx in (-1, 0, 1):
                out.append((r, d, kz, ky, kx))
                r += 16
    assert r == 736
    return out


_SLOTS = _slots()
_BRANCH = {0: 0, 6: 1, 12: 2, 18: 3}


def _pack_weights(w1, w2, w3, w4, wp, wdef, b1, b2, b3, b4, bdef):
    wb = {6: np.asarray(w2, np.float32), 12: np.asarray(w3, np.float32),
          18: np.asarray(w4, np.float32)}
    A1 = np.zeros((768, M1), np.float32)
    A1[0:16, 0:256] = np.asarray(w1, np.float32).reshape(256, 16).T
    for (r0, d, kz, ky, kx) in _SLOTS[1:]:
        A1[r0:r0 + 16, 256 * _BRANCH[d]:256 * (_BRANCH[d] + 1)] = \
            wb[d][:, :, kz + 1, ky + 1, kx + 1].T
    a1 = A1.reshape(6, 128, M1).transpose(1, 0, 2).reshape(128, 6 * M1)

    WpT = np.asarray(wp, np.float32).reshape(256, 1280)[:, :1024].T
    a2 = WpT.reshape(8, 128, 256).transpose(1, 0, 2).reshape(128, 8 * 256)

    A3 = np.asarray(wdef, np.float32).reshape(108, 256, 27) \
        .transpose(2, 1, 0).reshape(6912, 108)
    a3 = A3.reshape(54, 128, 108).transpose(1, 0, 2).reshape(128, 54 * 108)

    bias1 = np.concatenate([np.asarray(b, np.float32) for b in (b1, b2, b3, b4)])
    b1i = bias1.reshape(8, 128).T.copy()
    bdefi = np.zeros((128, 1), np.float32)
    bdefi[:108, 0] = np.asarray(bdef, np.float32)
    import ml_dtypes
    bf = ml_dtypes.bfloat16
    return {"a1": a1.astype(bf), "a2": a2.astype(bf), "a3": a3.astype(bf),
            "b1": b1i, "bdef": bdefi}


def _build_nc(pk):
    from contextlib import ExitStack
    import concourse.tile as tile
    from concourse import bacc, bass, mybir

    nc = bacc.Bacc("TRN2", target_bir_lowering=False, debug=False,
                   enable_asserts=False, num_devices=N_CORES)
    bf16 = mybir.dt.bfloat16
    f32 = mybir.dt.float32
    xin = nc.dram_tensor("xin", [CI, D, H, W], bf16, kind="ExternalInput").ap()
    b2in = nc.dram_tensor("b2in", [128, 2], f32, kind="ExternalInput").ap()
    gidx = nc.dram_tensor("gidx", [128, 6], mybir.dt.int32,
                          kind="ExternalInput").ap()
    defo = nc.dram_tensor("defo", [108, NPC], bf16, kind="ExternalOutput").ap()
    pyrd_h = nc.dram_tensor("pyrd", [2048, NPC], bf16, kind="Internal")
    pyrd = pyrd_h.ap()

    a1d = nc.inline_tensor(pk["a1"], "a1w").ap()
    a2d = nc.inline_tensor(pk["a2"], "a2w").ap()
    a3d = nc.inline_tensor(pk["a3"], "a3w").ap()
    b1d = nc.inline_tensor(pk["b1"], "b1w").ap()
    bdd = nc.inline_tensor(pk["bdef"], "bdw").ap()

    with tile.TileContext(nc) as tc:
        with ExitStack() as ctx:
            wpool = ctx.enter_context(tc.tile_pool(name="w", bufs=1))
            b1pool = ctx.enter_context(tc.tile_pool(name="b1", bufs=7))
            catpool = ctx.enter_context(tc.tile_pool(name="cat", bufs=9))
            pyrpool = ctx.enter_context(tc.tile_pool(name="pyr", bufs=4))
            slabpool = ctx.enter_context(tc.tile_pool(name="slab", bufs=6))
            b3pool = ctx.enter_context(tc.tile_pool(name="b3", bufs=3))
            opool = ctx.enter_context(tc.tile_pool(name="o", bufs=1))
            ps12 = ctx.enter_context(tc.tile_pool(name="ps12", bufs=3,
                                                  space="PSUM"))
            ps3 = ctx.enter_context(tc.tile_pool(name="ps3", bufs=5,
                                                 space="PSUM"))

            a1s = wpool.tile([128, 6 * M1], bf16, tag="a1s")
            a2s = wpool.tile([128, 8 * 256], bf16, tag="a2s")
            a3s = wpool.tile([128, 54 * 108], bf16, tag="a3s")
            b1s = wpool.tile([128, 8], f32, tag="b1s")
            b2s = wpool.tile([128, 2], f32, tag="b2s")
            bds = wpool.tile([128, 1], f32, tag="bds")
            gis = wpool.tile([128, 6], mybir.dt.int32, tag="gis")
            nc.sync.dma_start(a1s[:], a1d)
            nc.sync.dma_start(a2s[:], a2d)
            nc.sync.dma_start(a3s[:], a3d)
            nc.sync.dma_start(b1s[:], b1d)
            nc.sync.dma_start(b2s[:], b2in)
            nc.sync.dma_start(bds[:], bdd)
            nc.sync.dma_start(gis[:], gidx)
            a1v = a1s[:].rearrange("p (k m) -> p k m", k=6)
            a2v = a2s[:].rearrange("p (k m) -> p k m", k=8)
            a3v = a3s[:].rearrange("p (k m) -> p k m", k=54)

            for z in range(D):
                b1t = [b1pool.tile([128, NPC], bf16, tag="b1t", name="b1t")
                       for _ in range(K1T)]
                for t in b1t:
                    nc.vector.memset(t[:], 0)
                for (r0, d, kz, ky, kx) in _SLOTS:
                    zin = z + kz * d
                    if not (0 <= zin < D):
                        continue
                    ys, ye = max(0, -ky * d), H - max(0, ky * d)
                    xs, xe = max(0, -kx * d), W - max(0, kx * d)
                    if ys >= ye or xs >= xe:
                        continue
                    kt, po = divmod(r0, 128)
                    dst = b1t[kt][po:po + 16, :] \
                        .rearrange("p (y x) -> p y x", y=H)[:, ys:ye, xs:xe]
                    src = xin[:, zin, ys + ky * d:ye + ky * d,
                              xs + kx * d:xe + kx * d]
                    nc.sync.dma_start(dst, src)

                catt = [catpool.tile([128, NPC], bf16, tag="catt", name="catt")
                        for _ in range(K2T)]
                for mt in range(8):
                    for (n0, nw) in NCH:
                        ps = ps12.tile([128, 512], f32, tag="ps")
                        for kt in range(K1T):
                            nc.tensor.matmul(
                                ps[:, :nw],
                                a1v[:, kt, mt * 128:(mt + 1) * 128],
                                b1t[kt][:, n0:n0 + nw],
                                start=(kt == 0), stop=(kt == K1T - 1))
                        nc.scalar.activation(
                            catt[mt][:, n0:n0 + nw], ps[:, :nw],
                            mybir.ActivationFunctionType.Relu,
                            bias=b1s[:, mt:mt + 1], scale=1.0)

                for mt2 in range(2):
                    pyrt = pyrpool.tile([128, NPC], bf16, tag="pyrt")
                    for (n0, nw) in NCH:
                        ps = ps12.tile([128, 512], f32, tag="ps")
                        for kt in range(K2T):
                            nc.tensor.matmul(
                                ps[:, :nw],
                                a2v[:, kt, mt2 * 128:(mt2 + 1) * 128],
                                catt[kt][:, n0:n0 + nw],
                                start=(kt == 0), stop=(kt == K2T - 1))
                        nc.scalar.activation(
                            pyrt[:, n0:n0 + nw], ps[:, :nw],
                            mybir.ActivationFunctionType.Relu,
                            bias=b2s[:, mt2:mt2 + 1], scale=1.0)
                    nc.sync.dma_start(
                        pyrd[z * 256 + mt2 * 128:z * 256 + (mt2 + 1) * 128, :],
                        pyrt[:])

            # gather own z-1..z+1 pyramid slab (OOB rows remain zero)
            st = [slabpool.tile([128, NPC], bf16, tag="st", name="st")
                  for _ in range(6)]
            for s in range(6):
                nc.vector.memset(st[s][:], 0)
                nc.gpsimd.indirect_dma_start(
                    out=st[s][:], out_offset=None, in_=pyrd,
                    in_offset=bass.IndirectOffsetOnAxis(
                        ap=gis[:, s:s + 1], axis=0),
                    bounds_check=2047, oob_is_err=False)

            pst = [ps3.tile([128, 512], f32, tag="pst", name="pst")
                   for _ in range(5)]
            for t in range(27):
                kz, r = divmod(t, 9)
                ky, kx = divmod(r, 3)
                kz, ky, kx = kz - 1, ky - 1, kx - 1
                ys, ye = max(0, -ky), H - max(0, ky)
                xs, xe = max(0, -kx), W - max(0, kx)
                for ct in range(2):
                    ktg = 2 * t + ct
                    b3 = b3pool.tile([128, NPC], bf16, tag="b3")
                    if ky or kx:
                        nc.vector.memset(b3[:], 0)
                    dst = b3[:].rearrange("p (y x) -> p y x", y=H)[:, ys:ye, xs:xe]
                    src = st[(kz + 1) * 2 + ct][:] \
                        .rearrange("p (y x) -> p y x", y=H)[:, ys + ky:ye + ky,
                                                            xs + kx:xe + kx]
                    nc.vector.tensor_copy(dst, src)
                    for ci, (n0, nw) in enumerate(NCH):
                        nc.tensor.matmul(
                            pst[ci][:108, :nw], a3v[:, ktg, :108],
                            b3[:, n0:n0 + nw],
                            start=(ktg == 0), stop=(ktg == K3T - 1))
            dfs = opool.tile([128, NPC], bf16, tag="dfs")
            for ci, (n0, nw) in enumerate(NCH):
                nc.scalar.activation(
                    dfs[:108, n0:n0 + nw], pst[ci][:108, :nw],
                    mybir.ActivationFunctionType.Identity,
                    bias=bds[:108, 0:1], scale=1.0)
            nc.sync.dma_start(defo, dfs[:108, :])
    nc.compile()
    return nc


def _gather_indices():
    gis = []
    for i in range(N_CORES):
        gi = np.full((128, 6), 1 << 20, np.int32)
        for s in range(6):
            gz = i - 1 + s // 2
            if 0 <= gz < D:
                gi[:, s] = gz * 256 + (s % 2) * 128 + np.arange(128)
        gis.append(gi)
    return gis


def _make_runner(nc):
    """Cached jit(shard_map(bass_exec)) runner; mirrors
    bass2jax.run_bass_via_pjrt but traces/compiles once. Output
    zero-donation buffers are produced on device (no host traffic)."""
    import jax
    import jax.numpy as jnp
    from jax.sharding import Mesh, PartitionSpec, NamedSharding
    from jax.experimental.shard_map import shard_map
    from concourse import bass2jax, mybir

    bass2jax.install_neuronx_cc_hook()
    partition_name = (nc.partition_id_tensor.name
                      if nc.partition_id_tensor else None)
    assert nc.dbg_addr is None

    in_names, out_names, out_avals = [], [], []
    for alloc in nc.m.functions[0].allocations:
        if not isinstance(alloc, mybir.MemoryLocationSet):
            continue
        name = alloc.memorylocations[0].name
        if alloc.kind == "ExternalInput":
            if name != partition_name:
                in_names.append(name)
        elif alloc.kind == "ExternalOutput":
            out_names.append(name)
            out_avals.append(jax.core.ShapedArray(
                tuple(alloc.tensor_shape), mybir.dt.np(alloc.dtype)))
    n_params = len(in_names)
    n_outs = len(out_names)
    bind_in_names = tuple(in_names + out_names
                          + ([partition_name] if partition_name else []))

    def _body(*args):
        operands = list(args)
        if partition_name is not None:
            operands.append(bass2jax.partition_id_tensor())
        outs = bass2jax._bass_exec_p.bind(
            *operands,
            out_avals=tuple(out_avals),
            in_names=bind_in_names,
            out_names=tuple(out_names),
            lowering_input_output_aliases=(),
            sim_require_finite=True,
            sim_require_nnan=True,
            nc=nc,
        )
        return tuple(outs)

    devices = jax.devices()[:N_CORES]
    mesh = Mesh(np.asarray(devices), ("core",))
    spec = PartitionSpec("core")
    sharded = jax.jit(
        shard_map(_body, mesh=mesh,
                  in_specs=(spec,) * (n_params + n_outs),
                  out_specs=(spec,) * n_outs, check_rep=False),
        donate_argnums=tuple(range(n_params, n_params + n_outs)),
        keep_unused=True)
    zmakers = [
        jax.jit(
            (lambda av: lambda: jnp.zeros(
                (N_CORES * av.shape[0], *av.shape[1:]), av.dtype))(av),
            out_shardings=NamedSharding(mesh, spec))
        for av in out_avals]

    def run(in_map_global):
        args = [zm() for zm in zmakers]
        args = [in_map_global[n] for n in in_names] + args
        outs = sharded(*args)
        o = np.asarray(outs[0])
        return o.reshape(N_CORES, -1, o.shape[-1])

    return run


def _fake_device(x_bf, b2c, pk):
    """Numpy emulation of the device program, for layout validation."""
    A1 = pk["a1"].astype(np.float32).reshape(128, 6, M1) \
        .transpose(1, 0, 2).reshape(768, M1)
    A2 = pk["a2"].astype(np.float32).reshape(128, 8, 256) \
        .transpose(1, 0, 2).reshape(1024, 256)
    A3 = pk["a3"].astype(np.float32).reshape(128, 54, 108) \
        .transpose(1, 0, 2).reshape(6912, 108)
    bias1 = pk["b1"].T.reshape(1024)
    bdef = pk["bdef"][:108, 0]
    bp = b2c.T.reshape(256)
    x = x_bf.astype(np.float32)
    pyr = np.zeros((D, 256, NPC), np.float32)
    for z in range(D):
        B1 = np.zeros((768, NPC), np.float32)
        for (r0, d, kz, ky, kx) in _SLOTS:
            zin = z + kz * d
            if not (0 <= zin < D):
                continue
            ys, ye = max(0, -ky * d), H - max(0, ky * d)
            xs, xe = max(0, -kx * d), W - max(0, kx * d)
            blk = np.zeros((16, H, W), np.float32)
            blk[:, ys:ye, xs:xe] = x[:, zin, ys + ky * d:ye + ky * d,
                                     xs + kx * d:xe + kx * d]
            B1[r0:r0 + 16] = blk.reshape(16, NPC)
        cat = np.maximum(A1.T @ B1 + bias1[:, None], 0.0)
        pyr[z] = np.maximum(A2.T @ cat + bp[:, None], 0.0)
    defs = []
    for i in range(N_CORES):
        B3 = np.zeros((6912, NPC), np.float32)
        for t in range(27):
            kz, r = divmod(t, 9)
            ky, kx = divmod(r, 3)
            kz, ky, kx = kz - 1, ky - 1, kx - 1
            gz = i + kz
            if not (0 <= gz < D):
                continue
            ys, ye = max(0, -ky), H - max(0, ky)
            xs, xe = max(0, -kx), W - max(0, kx)
            blk = np.zeros((256, H, W), np.float32)
            blk[:, ys:ye, xs:xe] = pyr[gz].reshape(256, H, W)[
                :, ys + ky:ye + ky, xs + kx:xe + kx]
            B3[t * 256:(t + 1) * 256] = blk.reshape(256, NPC)
        defs.append(A3.T @ B3 + bdef[:, None])
    return np.stack(defs)


def _run_device(x_bf, b2c, pk):
    """-> defo [N_CORES, 108, NPC] float-ish (core i = z-slice i)."""
    if _FAKE:
        return _fake_device(x_bf, b2c, pk)
    if "nc" not in _STATE:
        _STATE["nc"] = _build_nc(pk)
    if _V1 or _STATE.get("v1"):
        from concourse.bass_utils import run_bass_kernel_spmd
        gis = _gather_indices()
        ins = [{"xin": x_bf, "b2in": b2c, "gidx": gis[i]}
               for i in range(N_CORES)]
        res = run_bass_kernel_spmd(_STATE["nc"], ins,
                                   core_ids=list(range(N_CORES)))
        return np.stack([np.asarray(res.results[i]["defo"], np.float32)
                         for i in range(N_CORES)])
    try:
        if "runner" not in _STATE:
            _STATE["runner"] = _make_runner(_STATE["nc"])
            _STATE["gidx_g"] = np.concatenate(_gather_indices(), axis=0)
        xg = np.concatenate([x_bf] * N_CORES, axis=0)
        bg = np.concatenate([b2c] * N_CORES, axis=0)
        return _STATE["runner"](
            {"xin": xg, "b2in": bg, "gidx": _STATE["gidx_g"]})
    except Exception:
        _STATE["v1"] = True
        _STATE.pop("runner", None)
        return _run_device(x_bf, b2c, pk)


def _make_sampler():
    """jax-CPU jitted: defo -> sigmoid/offsets -> trilinear modulated
    sampling -> final 432->32 GEMM."""
    import jax
    import jax.numpy as jnp

    def fn(xf, defo, wdcf, bdc):
        # xf [NPOS, CI] f32, defo [108, NPOS] f32, wdcf [27, CI, 32]
        off = defo[:81].reshape(27, 3, NPOS)
        alpha = jax.nn.sigmoid(defo[81:108])                 # [27, NPOS]
        zz, yy, xx = jnp.meshgrid(jnp.arange(D), jnp.arange(H),
                                  jnp.arange(W), indexing="ij")
        base = jnp.stack([zz.reshape(-1), yy.reshape(-1),
                          xx.reshape(-1)]).astype(jnp.float32)   # [3, NPOS]
        kg = jnp.stack(jnp.meshgrid(*([jnp.arange(-1, 2)] * 3),
                                    indexing="ij"), -1).reshape(27, 3)
        p = base[None] + kg[:, :, None].astype(jnp.float32) + off
        pz, py, px = p[:, 0], p[:, 1], p[:, 2]
        z0 = jnp.floor(pz); y0 = jnp.floor(py); x0 = jnp.floor(px)
        fz = pz - z0; fy = py - y0; fx = px - x0
        z0 = z0.astype(jnp.int32); y0 = y0.astype(jnp.int32)
        x0 = x0.astype(jnp.int32)
        out = jnp.zeros((32, NPOS), jnp.float32)
        for dz in (0, 1):
            for dy in (0, 1):
                for dx in (0, 1):
                    zi = z0 + dz; yi = y0 + dy; xi = x0 + dx
                    valid = ((zi >= 0) & (zi < D) & (yi >= 0) & (yi < H)
                             & (xi >= 0) & (xi < W))
                    wz = fz if dz else (1.0 - fz)
                    wy = fy if dy else (1.0 - fy)
                    wx = fx if dx else (1.0 - fx)
                    wgt = wz * wy * wx * valid * alpha       # [27, NPOS]
                    lin = (jnp.clip(zi, 0, D - 1) * H
                           + jnp.clip(yi, 0, H - 1)) * W \
                        + jnp.clip(xi, 0, W - 1)             # [27, NPOS]
                    val = xf[lin]                            # [27, NPOS, CI]
                    out = out + jnp.einsum(
                        "kpc,kco->op", val * wgt[..., None], wdcf,
                        preferred_element_type=jnp.float32)
        return out + bdc[:, None]

    return jax.jit(fn)


def kernel(x, w1, b1, w2, b2, w3, b3, w4, b4, wg, bg, wp, bp,
           wdef, bdef, wdc, bdc):
    import ml_dtypes
    import jax
    x = np.asarray(x, np.float32)
    fp = sum(float(np.sum(np.asarray(a))) for a in
             (w1, w2, w3, w4, wp, wdef, b1, b2, b3, b4, bdef))
    if _STATE.get("fp") != fp:
        _STATE.clear()
        _STATE["fp"] = fp
        _STATE["pk"] = _pack_weights(w1, w2, w3, w4, wp, wdef,
                                     b1, b2, b3, b4, bdef)
        _STATE["sampler"] = _make_sampler()
    pk = _STATE["pk"]

    g = x[0].mean(axis=(1, 2, 3))
    brg = np.maximum(np.asarray(bg, np.float32)
                     + np.asarray(wg, np.float32).reshape(256, 16) @ g, 0.0)
    bp2 = np.asarray(bp, np.float32) \
        + np.asarray(wp, np.float32).reshape(256, 1280)[:, 1024:] @ brg
    b2c = np.ascontiguousarray(bp2.reshape(2, 128).T)

    x_bf = x[0].astype(ml_dtypes.bfloat16)
    defs = _run_device(x_bf, b2c, pk)                       # [8, 108, NPC]
    defo = np.ascontiguousarray(
        np.asarray(defs, np.float32).transpose(1, 0, 2)).reshape(108, NPOS)

    xf = np.ascontiguousarray(x[0].transpose(1, 2, 3, 0)).reshape(NPOS, CI)
    wdcf = np.asarray(wdc, np.float32).reshape(32, 16, 27).transpose(2, 1, 0)
    cpu = jax.devices("cpu")[0]
    with jax.default_device(cpu):
        out = np.asarray(_STATE["sampler"](
            xf, defo, wdcf, np.asarray(bdc, np.float32)))
    return out.reshape(1, 32, D, H, W).astype(np.float32)


# revision 5
# speedup vs baseline: 78.9115x; 3.1802x over previous
"""ASPPModulatedDeformableC3D on 8 Trainium2 NeuronCores.

Single fused device dispatch computes ASPP (all dilated branches packed
into one K=736 GEMM; global-pool branch folded into the stage-2 bias),
the 1280->256 projection, and the 3x3x3 offset conv. Every core
redundantly computes the full pyramid (compute is ~2ms, wire is the
bottleneck), writes it to device DRAM, then indirect-DMA-gathers its own
z-slice +-1 slab (per-core index input; OOB rows stay zero, giving exact
conv z-padding) and produces defo[108, 2304] bf16 for its slice.

Weights are embedded in the NEFF via inline_tensor, so the warm-call
wire traffic is only x (bf16) + a tiny x-dependent bias in, defo out.
The dispatch goes through a cached jit(shard_map(bass_exec)) built once
(run_bass_kernel_spmd re-traces every call; that alone costs ~1s).
The data-dependent trilinear sampling and the final 432x32 GEMM run on
host under a jax-CPU jit (cheap; no efficient device gather at this
granularity).

KERNEL_FAKE_GEMM=1 emulates the device program in numpy.
KERNEL_V1=1 forces the run_bass_kernel_spmd dispatch path.
"""
import os

import numpy as np

N_CORES = 8
CI, D, H, W = 16, 8, 48, 48
NPC = H * W                  # 2304 positions per z-slice (one core each)
NPOS = D * NPC
MID = 256
M1 = 1024                    # cat rows (4 branches; global folded into bias2)
K1T = 6                      # stage-1 K tiles (736 rows used, 768 padded)
K2T = 8                      # stage-2 K tiles (1024)
K3T = 54                     # stage-3 K tiles (6912 = 27 taps * 256)
NCH = [(0, 512), (512, 512), (1024, 512), (1536, 512), (2048, 256)]

_FAKE = bool(int(os.environ.get("KERNEL_FAKE_GEMM", "0")))
_V1 = bool(int(os.environ.get("KERNEL_V1", "0")))
_STATE = {}


def _slots():
    """B1/A1 row layout: list of (row0, dil, kz, ky, kx). Slot 0 is the
    1x1 branch; d12/d18 kz=+-1 taps are always out of z-bounds (D=8) and
    are omitted entirely."""
    out = [(0, 0, 0, 0, 0)]
    r = 16
    for d, kz in [(6, 0), (12, 0), (18, 0), (6, -1), (6, 1)]:
        for ky in (-1, 0, 1):
            for kx in (-1, 0, 1):
                out.append((r, d, kz, ky, kx))
                r += 16
    assert r == 736
    return out


_SLOTS = _slots()
_BRANCH = {0: 0, 6: 1, 12: 2, 18: 3}


def _pack_weights(w1, w2, w3, w4, wp, wdef, b1, b2, b3, b4, bdef):
    wb = {6: np.asarray(w2, np.float32), 12: np.asarray(w3, np.float32),
          18: np.asarray(w4, np.float32)}
    A1 = np.zeros((768, M1), np.float32)
    A1[0:16, 0:256] = np.asarray(w1, np.float32).reshape(256, 16).T
    for (r0, d, kz, ky, kx) in _SLOTS[1:]:
        A1[r0:r0 + 16, 256 * _BRANCH[d]:256 * (_BRANCH[d] + 1)] = \
            wb[d][:, :, kz + 1, ky + 1, kx + 1].T
    a1 = A1.reshape(6, 128, M1).transpose(1, 0, 2).reshape(128, 6 * M1)

    WpT = np.asarray(wp, np.float32).reshape(256, 1280)[:, :1024].T
    a2 = WpT.reshape(8, 128, 256).transpose(1, 0, 2).reshape(128, 8 * 256)

    A3 = np.asarray(wdef, np.float32).reshape(108, 256, 27) \
        .transpose(2, 1, 0).reshape(6912, 108)
    a3 = A3.reshape(54, 128, 108).transpose(1, 0, 2).reshape(128, 54 * 108)

    bias1 = np.concatenate([np.asarray(b, np.float32) for b in (b1, b2, b3, b4)])
    b1i = bias1.reshape(8, 128).T.copy()
    bdefi = np.zeros((128, 1), np.float32)
    bdefi[:108, 0] = np.asarray(bdef, np.float32)
    import ml_dtypes
    bf = ml_dtypes.bfloat16
    return {"a1": a1.astype(bf), "a2": a2.astype(bf), "a3": a3.astype(bf),
            "b1": b1i, "bdef": bdefi}


def _build_nc(pk):
    from contextlib import ExitStack
    import concourse.tile as tile
    from concourse import bacc, bass, mybir

    nc = bacc.Bacc("TRN2", target_bir_lowering=False, debug=False,
                   enable_asserts=False, num_devices=N_CORES)
    bf16 = mybir.dt.bfloat16
    f32 = mybir.dt.float32
    xin = nc.dram_tensor("xin", [CI, D, H, W], bf16, kind="ExternalInput").ap()
    b2in = nc.dram_tensor("b2in", [128, 2], f32, kind="ExternalInput").ap()
    gidx = nc.dram_tensor("gidx", [128, 6], mybir.dt.int32,
                          kind="ExternalInput").ap()
    defo = nc.dram_tensor("defo", [108, NPC], bf16, kind="ExternalOutput").ap()
    pyrd_h = nc.dram_tensor("pyrd", [2048, NPC], bf16, kind="Internal")
    pyrd = pyrd_h.ap()

    a1d = nc.inline_tensor(pk["a1"], "a1w").ap()
    a2d = nc.inline_tensor(pk["a2"], "a2w").ap()
    a3d = nc.inline_tensor(pk["a3"], "a3w").ap()
    b1d = nc.inline_tensor(pk["b1"], "b1w").ap()
    bdd = nc.inline_tensor(pk["bdef"], "bdw").ap()

    with tile.TileContext(nc) as tc:
        with ExitStack() as ctx:
            wpool = ctx.enter_context(tc.tile_pool(name="w", bufs=1))
            b1pool = ctx.enter_context(tc.tile_pool(name="b1", bufs=7))
            catpool = ctx.enter_context(tc.tile_pool(name="cat", bufs=9))
            pyrpool = ctx.enter_context(tc.tile_pool(name="pyr", bufs=4))
            slabpool = ctx.enter_context(tc.tile_pool(name="slab", bufs=6))
            b3pool = ctx.enter_context(tc.tile_pool(name="b3", bufs=3))
            opool = ctx.enter_context(tc.tile_pool(name="o", bufs=1))
            ps12 = ctx.enter_context(tc.tile_pool(name="ps12", bufs=3,
                                                  space="PSUM"))
            ps3 = ctx.enter_context(tc.tile_pool(name="ps3", bufs=5,
                                                 space="PSUM"))

            a1s = wpool.tile([128, 6 * M1], bf16, tag="a1s")
            a2s = wpool.tile([128, 8 * 256], bf16, tag="a2s")
            a3s = wpool.tile([128, 54 * 108], bf16, tag="a3s")
            b1s = wpool.tile([128, 8], f32, tag="b1s")
            b2s = wpool.tile([128, 2], f32, tag="b2s")
            bds = wpool.tile([128, 1], f32, tag="bds")
            gis = wpool.tile([128, 6], mybir.dt.int32, tag="gis")
            nc.sync.dma_start(a1s[:], a1d)
            nc.sync.dma_start(a2s[:], a2d)
            nc.sync.dma_start(a3s[:], a3d)
            nc.sync.dma_start(b1s[:], b1d)
            nc.sync.dma_start(b2s[:], b2in)
            nc.sync.dma_start(bds[:], bdd)
            nc.sync.dma_start(gis[:], gidx)
            a1v = a1s[:].rearrange("p (k m) -> p k m", k=6)
            a2v = a2s[:].rearrange("p (k m) -> p k m", k=8)
            a3v = a3s[:].rearrange("p (k m) -> p k m", k=54)

            for z in range(D):
                b1t = [b1pool.tile([128, NPC], bf16, tag="b1t", name="b1t")
                       for _ in range(K1T)]
                for t in b1t:
                    nc.vector.memset(t[:], 0)
                for (r0, d, kz, ky, kx) in _SLOTS:
                    zin = z + kz * d
                    if not (0 <= zin < D):
                        continue
                    ys, ye = max(0, -ky * d), H - max(0, ky * d)
                    xs, xe = max(0, -kx * d), W - max(0, kx * d)
                    if ys >= ye or xs >= xe:
                        continue
                    kt, po = divmod(r0, 128)
                    dst = b1t[kt][po:po + 16, :] \
                        .rearrange("p (y x) -> p y x", y=H)[:, ys:ye, xs:xe]
                    src = xin[:, zin, ys + ky * d:ye + ky * d,
                              xs + kx * d:xe + kx * d]
                    nc.sync.dma_start(dst, src)

                catt = [catpool.tile([128, NPC], bf16, tag="catt", name="catt")
                        for _ in range(K2T)]
                for mt in range(8):
                    for (n0, nw) in NCH:
                        ps = ps12.tile([128, 512], f32, tag="ps")
                        for kt in range(K1T):
                            nc.tensor.matmul(
                                ps[:, :nw],
                                a1v[:, kt, mt * 128:(mt + 1) * 128],
                                b1t[kt][:, n0:n0 + nw],
                                start=(kt == 0), stop=(kt == K1T - 1))
                        nc.scalar.activation(
                            catt[mt][:, n0:n0 + nw], ps[:, :nw],
                            mybir.ActivationFunctionType.Relu,
                            bias=b1s[:, mt:mt + 1], scale=1.0)

                for mt2 in range(2):
                    pyrt = pyrpool.tile([128, NPC], bf16, tag="pyrt")
                    for (n0, nw) in NCH:
                        ps = ps12.tile([128, 512], f32, tag="ps")
                        for kt in range(K2T):
                            nc.tensor.matmul(
                                ps[:, :nw],
                                a2v[:, kt, mt2 * 128:(mt2 + 1) * 128],
                                catt[kt][:, n0:n0 + nw],
                                start=(kt == 0), stop=(kt == K2T - 1))
                        nc.scalar.activation(
                            pyrt[:, n0:n0 + nw], ps[:, :nw],
                            mybir.ActivationFunctionType.Relu,
                            bias=b2s[:, mt2:mt2 + 1], scale=1.0)
                    nc.sync.dma_start(
                        pyrd[z * 256 + mt2 * 128:z * 256 + (mt2 + 1) * 128, :],
                        pyrt[:])

            # gather own z-1..z+1 pyramid slab (OOB rows remain zero)
            st = [slabpool.tile([128, NPC], bf16, tag="st", name="st")
                  for _ in range(6)]
            for s in range(6):
                nc.vector.memset(st[s][:], 0)
                nc.gpsimd.indirect_dma_start(
                    out=st[s][:], out_offset=None, in_=pyrd,
                    in_offset=bass.IndirectOffsetOnAxis(
                        ap=gis[:, s:s + 1], axis=0),
                    bounds_check=2047, oob_is_err=False)

            pst = [ps3.tile([128, 512], f32, tag="pst", name="pst")
                   for _ in range(5)]
            for t in range(27):
                kz, r = divmod(t, 9)
                ky, kx = divmod(r, 3)
                kz, ky, kx = kz - 1, ky - 1, kx - 1
                ys, ye = max(0, -ky), H - max(0, ky)
                xs, xe = max(0, -kx), W - max(0, kx)
                for ct in range(2):
                    ktg = 2 * t + ct
                    b3 = b3pool.tile([128, NPC], bf16, tag="b3")
                    if ky or kx:
                        nc.vector.memset(b3[:], 0)
                    dst = b3[:].rearrange("p (y x) -> p y x", y=H)[:, ys:ye, xs:xe]
                    src = st[(kz + 1) * 2 + ct][:] \
                        .rearrange("p (y x) -> p y x", y=H)[:, ys + ky:ye + ky,
                                                            xs + kx:xe + kx]
                    nc.vector.tensor_copy(dst, src)
                    for ci, (n0, nw) in enumerate(NCH):
                        nc.tensor.matmul(
                            pst[ci][:108, :nw], a3v[:, ktg, :108],
                            b3[:, n0:n0 + nw],
                            start=(ktg == 0), stop=(ktg == K3T - 1))
            dfs = opool.tile([128, NPC], bf16, tag="dfs")
            for ci, (n0, nw) in enumerate(NCH):
                nc.scalar.activation(
                    dfs[:108, n0:n0 + nw], pst[ci][:108, :nw],
                    mybir.ActivationFunctionType.Identity,
                    bias=bds[:108, 0:1], scale=1.0)
            nc.sync.dma_start(defo, dfs[:108, :])
    nc.compile()
    return nc


def _gather_indices():
    gis = []
    for i in range(N_CORES):
        gi = np.full((128, 6), 1 << 20, np.int32)
        for s in range(6):
            gz = i - 1 + s // 2
            if 0 <= gz < D:
                gi[:, s] = gz * 256 + (s % 2) * 128 + np.arange(128)
        gis.append(gi)
    return gis


def _make_runner(nc):
    """Cached jit(shard_map(bass_exec)) runner; mirrors
    bass2jax.run_bass_via_pjrt but traces/compiles once. Output
    zero-donation buffers are produced on device (no host traffic)."""
    import jax
    import jax.numpy as jnp
    from jax.sharding import Mesh, PartitionSpec, NamedSharding
    from jax.experimental.shard_map import shard_map
    from concourse import bass2jax, mybir

    bass2jax.install_neuronx_cc_hook()
    partition_name = (nc.partition_id_tensor.name
                      if nc.partition_id_tensor else None)
    assert nc.dbg_addr is None

    in_names, out_names, out_avals = [], [], []
    for alloc in nc.m.functions[0].allocations:
        if not isinstance(alloc, mybir.MemoryLocationSet):
            continue
        name = alloc.memorylocations[0].name
        if alloc.kind == "ExternalInput":
            if name != partition_name:
                in_names.append(name)
        elif alloc.kind == "ExternalOutput":
            out_names.append(name)
            out_avals.append(jax.core.ShapedArray(
                tuple(alloc.tensor_shape), mybir.dt.np(alloc.dtype)))
    n_params = len(in_names)
    n_outs = len(out_names)
    bind_in_names = tuple(in_names + out_names
                          + ([partition_name] if partition_name else []))

    def _body(*args):
        operands = list(args)
        if partition_name is not None:
            operands.append(bass2jax.partition_id_tensor())
        outs = bass2jax._bass_exec_p.bind(
            *operands,
            out_avals=tuple(out_avals),
            in_names=bind_in_names,
            out_names=tuple(out_names),
            lowering_input_output_aliases=(),
            sim_require_finite=True,
            sim_require_nnan=True,
            nc=nc,
        )
        return tuple(outs)

    devices = jax.devices()[:N_CORES]
    mesh = Mesh(np.asarray(devices), ("core",))
    spec = PartitionSpec("core")
    sharded = jax.jit(
        shard_map(_body, mesh=mesh,
                  in_specs=(spec,) * (n_params + n_outs),
                  out_specs=(spec,) * n_outs, check_rep=False),
        donate_argnums=tuple(range(n_params, n_params + n_outs)),
        keep_unused=True)
    zmakers = [
        jax.jit(
            (lambda av: lambda: jnp.zeros(
                (N_CORES * av.shape[0], *av.shape[1:]), av.dtype))(av),
            out_shardings=NamedSharding(mesh, spec))
        for av in out_avals]

    def run(in_map_global):
        args = [zm() for zm in zmakers]
        args = [in_map_global[n] for n in in_names] + args
        outs = sharded(*args)
        o = np.asarray(outs[0])
        return o.reshape(N_CORES, -1, o.shape[-1])

    return run


def _fake_device(x_bf, b2c, pk):
    """Numpy emulation of the device program, for layout validation."""
    A1 = pk["a1"].astype(np.float32).reshape(128, 6, M1) \
        .transpose(1, 0, 2).reshape(768, M1)
    A2 = pk["a2"].astype(np.float32).reshape(128, 8, 256) \
        .transpose(1, 0, 2).reshape(1024, 256)
    A3 = pk["a3"].astype(np.float32).reshape(128, 54, 108) \
        .transpose(1, 0, 2).reshape(6912, 108)
    bias1 = pk["b1"].T.reshape(1024)
    bdef = pk["bdef"][:108, 0]
    bp = b2c.T.reshape(256)
    x = x_bf.astype(np.float32)
    pyr = np.zeros((D, 256, NPC), np.float32)
    for z in range(D):
        B1 = np.zeros((768, NPC), np.float32)
        for (r0, d, kz, ky, kx) in _SLOTS:
            zin = z + kz * d
            if not (0 <= zin < D):
                continue
            ys, ye = max(0, -ky * d), H - max(0, ky * d)
            xs, xe = max(0, -kx * d), W - max(0, kx * d)
            blk = np.zeros((16, H, W), np.float32)
            blk[:, ys:ye, xs:xe] = x[:, zin, ys + ky * d:ye + ky * d,
                                     xs + kx * d:xe + kx * d]
            B1[r0:r0 + 16] = blk.reshape(16, NPC)
        cat = np.maximum(A1.T @ B1 + bias1[:, None], 0.0)
        pyr[z] = np.maximum(A2.T @ cat + bp[:, None], 0.0)
    defs = []
    for i in range(N_CORES):
        B3 = np.zeros((6912, NPC), np.float32)
        for t in range(27):
            kz, r = divmod(t, 9)
            ky, kx = divmod(r, 3)
            kz, ky, kx = kz - 1, ky - 1, kx - 1
            gz = i + kz
            if not (0 <= gz < D):
                continue
            ys, ye = max(0, -ky), H - max(0, ky)
            xs, xe = max(0, -kx), W - max(0, kx)
            blk = np.zeros((256, H, W), np.float32)
            blk[:, ys:ye, xs:xe] = pyr[gz].reshape(256, H, W)[
                :, ys + ky:ye + ky, xs + kx:xe + kx]
            B3[t * 256:(t + 1) * 256] = blk.reshape(256, NPC)
        defs.append(A3.T @ B3 + bdef[:, None])
    return np.stack(defs)


def _run_device(x_bf, b2c, pk):
    """-> defo [N_CORES, 108, NPC] float-ish (core i = z-slice i)."""
    if _FAKE:
        return _fake_device(x_bf, b2c, pk)
    if "nc" not in _STATE:
        _STATE["nc"] = _build_nc(pk)
    if _V1 or _STATE.get("v1"):
        from concourse.bass_utils import run_bass_kernel_spmd
        gis = _gather_indices()
        ins = [{"xin": x_bf, "b2in": b2c, "gidx": gis[i]}
               for i in range(N_CORES)]
        res = run_bass_kernel_spmd(_STATE["nc"], ins,
                                   core_ids=list(range(N_CORES)))
        return np.stack([np.asarray(res.results[i]["defo"], np.float32)
                         for i in range(N_CORES)])
    try:
        if "runner" not in _STATE:
            _STATE["runner"] = _make_runner(_STATE["nc"])
            import jax
            from jax.sharding import Mesh, PartitionSpec, NamedSharding
            mesh = Mesh(np.asarray(jax.devices()[:N_CORES]), ("core",))
            _STATE["gidx_g"] = jax.device_put(
                np.concatenate(_gather_indices(), axis=0),
                NamedSharding(mesh, PartitionSpec("core")))
        xg = np.concatenate([x_bf] * N_CORES, axis=0)
        bg = np.concatenate([b2c] * N_CORES, axis=0)
        return _STATE["runner"](
            {"xin": xg, "b2in": bg, "gidx": _STATE["gidx_g"]})
    except Exception:
        _STATE["v1"] = True
        _STATE.pop("runner", None)
        return _run_device(x_bf, b2c, pk)


def _make_sampler():
    """Fused single-pass trilinear modulated sampling (numba; single-core
    host). Returns fn(xf [NPOS,CI], defo [8,108,NPC] f32) -> col
    [27, NPOS, CI] with col = alpha_k * sample_k."""
    import numba

    @numba.njit(fastmath=True, cache=False)
    def samp(xf, defo, col):
        for k in range(27):
            kz = k // 9 - 1
            ky = (k // 3) % 3 - 1
            kx = k % 3 - 1
            for p in range(NPOS):
                bz = p // NPC
                pp = p % NPC
                by = pp // W
                bx = pp % W
                a = 1.0 / (1.0 + np.exp(-defo[bz, 81 + k, pp]))
                pz = bz + kz + defo[bz, 3 * k + 0, pp]
                py = by + ky + defo[bz, 3 * k + 1, pp]
                px = bx + kx + defo[bz, 3 * k + 2, pp]
                z0 = int(np.floor(pz))
                y0 = int(np.floor(py))
                x0 = int(np.floor(px))
                fz = pz - z0
                fy = py - y0
                fx = px - x0
                acc = np.zeros(CI, np.float32)
                for dz in range(2):
                    zi = z0 + dz
                    if zi < 0 or zi >= D:
                        continue
                    wz = fz if dz else 1.0 - fz
                    for dy in range(2):
                        yi = y0 + dy
                        if yi < 0 or yi >= H:
                            continue
                        wy = wz * (fy if dy else 1.0 - fy)
                        for dx in range(2):
                            xi = x0 + dx
                            if xi < 0 or xi >= W:
                                continue
                            w = wy * (fx if dx else 1.0 - fx)
                            src = (zi * H + yi) * W + xi
                            for c in range(CI):
                                acc[c] += w * xf[src, c]
                for c in range(CI):
                    col[k, p, c] = a * acc[c]

    return samp


def _sample_numpy(xf, defo, col):
    """Numpy fallback (no numba): same contract as the numba sampler."""
    df = np.ascontiguousarray(
        np.asarray(defo, np.float32).transpose(1, 0, 2)).reshape(108, NPOS)
    off = df[:81].reshape(27, 3, NPOS)
    alpha = 1.0 / (1.0 + np.exp(-df[81:108]))
    zz, yy, xx = np.meshgrid(np.arange(D), np.arange(H), np.arange(W),
                             indexing="ij")
    base = np.stack([zz.ravel(), yy.ravel(), xx.ravel()]).astype(np.float32)
    kg = np.stack(np.meshgrid(*([np.arange(-1, 2)] * 3), indexing="ij"), -1)
    p = base[None] + kg.reshape(27, 3).astype(np.float32)[:, :, None] + off
    pz, py, px = p[:, 0], p[:, 1], p[:, 2]
    z0 = np.floor(pz); y0 = np.floor(py); x0 = np.floor(px)
    fz = pz - z0; fy = py - y0; fx = px - x0
    z0 = z0.astype(np.int64); y0 = y0.astype(np.int64); x0 = x0.astype(np.int64)
    acc = np.zeros((27, NPOS, CI), np.float32)
    for dz in (0, 1):
        for dy in (0, 1):
            for dx in (0, 1):
                zi = z0 + dz; yi = y0 + dy; xi = x0 + dx
                valid = ((zi >= 0) & (zi < D) & (yi >= 0) & (yi < H)
                         & (xi >= 0) & (xi < W))
                wz = fz if dz else (1.0 - fz)
                wy = fy if dy else (1.0 - fy)
                wx = fx if dx else (1.0 - fx)
                wgt = (wz * wy * wx * valid).astype(np.float32)
                lin = (np.clip(zi, 0, D - 1) * H + np.clip(yi, 0, H - 1)) * W \
                    + np.clip(xi, 0, W - 1)
                acc += xf[lin] * wgt[..., None]
    acc *= alpha[..., None]
    return acc


def kernel(x, w1, b1, w2, b2, w3, b3, w4, b4, wg, bg, wp, bp,
           wdef, bdef, wdc, bdc):
    import ml_dtypes
    x = np.asarray(x, np.float32)
    fp = sum(float(np.sum(np.asarray(a))) for a in
             (w1, w2, w3, w4, wp, wdef, b1, b2, b3, b4, bdef))
    if _STATE.get("fp") != fp:
        _STATE.clear()
        _STATE["fp"] = fp
        _STATE["pk"] = _pack_weights(w1, w2, w3, w4, wp, wdef,
                                     b1, b2, b3, b4, bdef)
        try:
            _STATE["sampler"] = _make_sampler()
        except Exception:
            _STATE["sampler"] = None
    pk = _STATE["pk"]

    g = x[0].mean(axis=(1, 2, 3))
    brg = np.maximum(np.asarray(bg, np.float32)
                     + np.asarray(wg, np.float32).reshape(256, 16) @ g, 0.0)
    bp2 = np.asarray(bp, np.float32) \
        + np.asarray(wp, np.float32).reshape(256, 1280)[:, 1024:] @ brg
    b2c = np.ascontiguousarray(bp2.reshape(2, 128).T)

    x_bf = x[0].astype(ml_dtypes.bfloat16)
    defs = _run_device(x_bf, b2c, pk)                 # [8, 108, NPC]
    defo = np.ascontiguousarray(np.asarray(defs, np.float32))

    xf = np.ascontiguousarray(x[0].transpose(1, 2, 3, 0)).reshape(NPOS, CI)
    wdcf = np.asarray(wdc, np.float32).reshape(32, 16, 27).transpose(2, 1, 0)
    wdcf = np.ascontiguousarray(wdcf)
    if _STATE["sampler"] is not None:
        col = np.empty((27, NPOS, CI), np.float32)
        _STATE["sampler"](xf, defo, col)
    else:
        col = _sample_numpy(xf, defo, None)
    out = np.einsum("kpc,kco->op", col, wdcf, optimize=True) \
        + np.asarray(bdc, np.float32)[:, None]
    return out.reshape(1, 32, D, H, W).astype(np.float32)
